# revision 13
# baseline (speedup 1.0000x reference)
"""NeuroSAT message-passing kernel for 8 Trainium2 NeuronCores (Bass/Tile).

Strategy
--------
A = D_row @ B @ D_col with B binary; B streams/resides in fp8 as the moving
matmul operand against bf16/fp8 stationary message tiles. Degree scalings
are applied at PSUM eviction; the last-layer MLP biases are folded in as
precomputed rank-1 outer-product tiles (G1/G2) added at eviction.

Sharding (8 cores):
  - clauses: core k owns [2048k, 2048k+2048)
  - literals: core k owns [512k, 512k+512) u [4096+512k, 4096+512k+512)

Cross-core exchange of the fp8 message images (lpre/cpre) is done with
direct SBUF->SBUF remote DMAs (remote_dma_broadcast, one real dest per
instruction) instead of ring collectives: core r stores peer c's block at
slot d = c XOR r, so the send APs are compile-time constants under SPMD;
each core's B slabs are host-permuted into its own XOR slot order. Waits
on remote arrival are explicit semaphore waits on the consuming engine;
the contraction loops consume the local slot first so the wire latency
hides under local matmuls. b1 (A^T direction) slots 0-5 stay resident in
SBUF; b1 slots 6-7 and all of b2 stream per round through a small rotating
buffer pool.
"""
import sys

sys.path.insert(0, "/opt/trn_rl_repo")

import numpy as np
import ml_dtypes

import concourse.bass as bass
import concourse.bass_interp as bass_interp
import concourse.mybir as mybir
import concourse.tile as tile
from concourse import bacc
from concourse import bass_utils
from concourse.bass import create_sync_update

dt = mybir.dt
AF = mybir.ActivationFunctionType
ALU = mybir.AluOpType
bf16 = ml_dtypes.bfloat16
f8 = ml_dtypes.float8_e4m3

NCORES = 8
D = 128
NL_TOT, NCL_TOT, NV = 8192, 16384, 4096
NL = NL_TOT // NCORES      # 1024 lits per core
NCL = NCL_TOT // NCORES    # 2048 clauses per core
PIN = 6                    # b1 slots resident in SBUF
FP8_ONE = 0x38             # bit pattern of 1.0 in float8_e4m3
GAIN = np.float32(128.0)   # power-of-2 pre-scale keeping fp8 messages normal


# Semaphores bumped by REMOTE cores are invisible to the single-core Tile
# scheduling simulator; pre-satisfy them there (build-time only — the
# emitted program keeps the real waits, and HW execution is unaffected).
_EXT_SEMS = []
_orig_simulate = bass_interp.CoreSim.simulate


def _patched_simulate(self, *a, **kw):
    if getattr(self, "scheduling_pass", False):
        for sem in _EXT_SEMS:
            self.update_semaphore(create_sync_update(sem, 1 << 20, None, True))
    return _orig_simulate(self, *a, **kw)


bass_interp.CoreSim.simulate = _patched_simulate


def _rdest(d):
    # Driver's logical->physical NC map flips bit 1 on the far die; the
    # relative XOR happens in physical tpb space. Uniform across cores.
    return (0, d ^ 2 if d & 4 else d)


# ---------------------------------------------------------------------------
# device program
# ---------------------------------------------------------------------------

def build_program(rounds: int):
    nc = bacc.Bacc("TRN2", target_bir_lowering=False, debug=False,
                   num_devices=NCORES)

    def inp(name, shape, dty):
        return nc.dram_tensor(name, list(shape), dty, kind="ExternalInput")

    b1 = inp("b1", [NCORES, 2, 128, 8192], dt.float8e4)
    b2 = inp("b2", [NCORES, 2, 128, 8192], dt.float8e4)
    w = {}
    for p in ("lm", "cm", "lv"):
        for l in ("w1t", "w2t", "w3t"):
            shape = [128, 1] if (p, l) == ("lv", "w3t") else [128, 128]
            w[f"{p}_{l}"] = inp(f"{p}_{l}", shape, dt.bfloat16)
        for l in ("b1", "b2"):
            w[f"{p}_{l}"] = inp(f"{p}_{l}", [128, 1], dt.float32)
    cu_wt = inp("cu_wt", [128, 512], dt.bfloat16)
    cu_ut = inp("cu_ut", [128, 512], dt.bfloat16)
    cu_b = inp("cu_b", [128, 4], dt.float32)
    lu_wcl = inp("lu_wcl", [128, 512], dt.bfloat16)
    lu_wfl = inp("lu_wfl", [128, 512], dt.bfloat16)
    lu_ut = inp("lu_ut", [128, 512], dt.bfloat16)
    lu_b = inp("lu_b", [128, 4], dt.float32)
    colb = inp("colb", [128, NCL], dt.bfloat16)
    rowb = inp("rowb", [128, NL], dt.bfloat16)
    g1t = inp("g1t", [128, NCL], dt.bfloat16)
    g2t = inp("g2t", [128, NL], dt.bfloat16)
    rowsc = inp("rowsc", [128, 8], dt.float32)
    colsc = inp("colsc", [128, 16], dt.float32)
    lh0 = inp("lh0", [128, NL], dt.bfloat16)
    ch0 = inp("ch0", [128, NCL], dt.bfloat16)
    vote_out = nc.dram_tensor("vote", [1, NL], dt.float32,
                              kind="ExternalOutput")

    ag1_sem = nc.alloc_semaphore("ag1_sem")
    ag2_sem = nc.alloc_semaphore("ag2_sem")
    loc1_sem = nc.alloc_semaphore("loc1_sem")
    loc2_sem = nc.alloc_semaphore("loc2_sem")
    _EXT_SEMS.clear()
    _EXT_SEMS.extend([ag1_sem, ag2_sem, loc1_sem, loc2_sem])

    with tile.TileContext(nc) as tc:
        with (
            tc.tile_pool(name="const", bufs=1) as cp,
            tc.tile_pool(name="state", bufs=1) as sp,
            tc.tile_pool(name="work", bufs=1) as wp,
            tc.tile_pool(name="chunk", bufs=1) as kp,
            tc.tile_pool(name="stream", bufs=3) as st,
            tc.tile_pool(name="psd", bufs=1, space="PSUM") as psd,
            tc.tile_pool(name="psg", bufs=1, space="PSUM") as psg,
            tc.tile_pool(name="psm", bufs=2, space="PSUM") as psm,
        ):
            # ---- constants into SBUF ----
            C = {}
            for name, t in [("cu_wt", cu_wt), ("cu_ut", cu_ut),
                            ("lu_wcl", lu_wcl), ("lu_wfl", lu_wfl),
                            ("lu_ut", lu_ut)]:
                C[name] = cp.tile([128, 512], dt.bfloat16, name=name)
                nc.scalar.dma_start(out=C[name], in_=t.ap())
            for p in ("lm", "cm", "lv"):
                for l in ("w1t", "w2t", "w3t"):
                    shape = [128, 1] if (p, l) == ("lv", "w3t") else [128, 128]
                    C[f"{p}_{l}"] = cp.tile(shape, dt.bfloat16, name=f"{p}_{l}")
                    nc.scalar.dma_start(out=C[f"{p}_{l}"], in_=w[f"{p}_{l}"].ap())
                for l in ("b1", "b2"):
                    C[f"{p}_{l}"] = cp.tile([128, 1], dt.float32,
                                            name=f"{p}_{l}")
                    nc.scalar.dma_start(out=C[f"{p}_{l}"], in_=w[f"{p}_{l}"].ap())
            for name, t, shape, dty in [
                ("cu_b", cu_b, [128, 4], dt.float32),
                ("lu_b", lu_b, [128, 4], dt.float32),
                ("colb", colb, [128, NCL], dt.bfloat16),
                ("rowb", rowb, [128, NL], dt.bfloat16),
                ("g1t", g1t, [128, NCL], dt.bfloat16),
                ("g2t", g2t, [128, NL], dt.bfloat16),
                ("rowsc", rowsc, [128, 8], dt.float32),
                ("colsc", colsc, [128, 16], dt.float32),
            ]:
                C[name] = cp.tile(shape, dty, name=name)
                nc.scalar.dma_start(out=C[name], in_=t.ap())

            # ---- states ----
            Lh_pp = [sp.tile([128, NL], dt.bfloat16, name="Lh_a"),
                     sp.tile([128, NL], dt.bfloat16, name="Lh_b")]
            Ch = sp.tile([128, NCL], dt.bfloat16, name="Ch")
            Lc = sp.tile([128, NL], dt.float32, name="Lc")
            Cc = sp.tile([128, NCL], dt.bfloat16, name="Cc")
            nc.scalar.dma_start(out=Lh_pp[0], in_=lh0.ap())
            nc.scalar.dma_start(out=Ch, in_=ch0.ap())
            nc.vector.memset(Lc, 0.0)
            nc.vector.memset(Cc, 0.0)

            # ---- resident b1 slots ----
            b1pin = []
            for d in range(PIN):
                t = cp.tile([128, 16384], dt.float8e4, name=f"b1pin{d}")
                nc.sync.dma_start(out=t[:, 0:8192], in_=b1.ap()[d, 0])
                nc.sync.dma_start(out=t[:, 8192:16384], in_=b1.ap()[d, 1])
                b1pin.append(t)

            # ---- message-exchange buffers (XOR slot order) ----
            lpre_full = wp.tile([128, NL_TOT], dt.float8e4, name="lpre_full")
            cpre_full = wp.tile([128, NCL_TOT], dt.float8e4, name="cpre_full")

            def ag_send(full, slotw, off, width, sem_r, sem_l):
                for d in range(1, NCORES):
                    rd = [None] * NCORES
                    rd[d] = _rdest(d)
                    nc.gpsimd.remote_dma_broadcast(
                        out_ap=full[:, d * slotw + off:d * slotw + off + width],
                        in_ap=full[:, off:off + width],
                        remote_sem=sem_r, local_sem=sem_l, rdests=rd)
                nc.gpsimd.trigger_dma(count=None)

            def l_msg_chunk(Lh_src, m, batches):
                """L message for my lit chunk m into lpre_full slot 0."""
                sl = slice(512 * m, 512 * (m + 1))
                ps = psm.tile([128, 512], dt.float32, tag="m", name=f"lm1_{m}")
                nc.tensor.matmul(ps, C["lm_w1t"], Lh_src[:, sl],
                                 start=True, stop=True)
                h1 = kp.tile([128, 512], dt.bfloat16, tag="h1", bufs=2,
                             name=f"lh1_{m}")
                nc.scalar.activation(h1, ps, AF.Relu, bias=C["lm_b1"])
                ps = psm.tile([128, 512], dt.float32, tag="m", name=f"lm2_{m}")
                nc.tensor.matmul(ps, C["lm_w2t"], h1, start=True, stop=True)
                h2 = kp.tile([128, 512], dt.bfloat16, tag="h2", bufs=2,
                             name=f"lh2_{m}")
                nc.scalar.activation(h2, ps, AF.Relu, bias=C["lm_b2"])
                for jj in range(4):
                    t = 4 * m + jj
                    ps = psm.tile([128, 128], dt.float32, tag="m",
                                  name=f"lm3_{m}_{jj}")
                    nc.tensor.matmul(ps, h2[:, 128 * jj:128 * (jj + 1)],
                                     C["lm_w3t"], start=True, stop=True)
                    act = nc.scalar.activation(
                        lpre_full[:, 128 * t:128 * (t + 1)], ps, AF.Copy,
                        scale=C["rowsc"][:, t:t + 1])
                    if batches > 0:
                        # slot-0 overwrite gated on prior sends draining
                        act.wait_op(loc1_sem, 112 * batches, "sem-ge")
                if m == 1:
                    # one fat send of the whole slot-0 lpre per peer: dummy
                    # lanes emit descs per broadcast, so fewer+fatter wins
                    ag_send(lpre_full, NL, 0, NL, ag1_sem, loc1_sem)

            def c_msg_chunk(nn, batches):
                """C message for my clause chunk nn into cpre_full slot 0."""
                sl = slice(512 * nn, 512 * (nn + 1))
                ps = psm.tile([128, 512], dt.float32, tag="m", name=f"cm1_{nn}")
                nc.tensor.matmul(ps, C["cm_w1t"], Ch[:, sl],
                                 start=True, stop=True)
                h1 = kp.tile([128, 512], dt.bfloat16, tag="h1", bufs=2,
                             name=f"ch1_{nn}")
                nc.scalar.activation(h1, ps, AF.Relu, bias=C["cm_b1"])
                ps = psm.tile([128, 512], dt.float32, tag="m", name=f"cm2_{nn}")
                nc.tensor.matmul(ps, C["cm_w2t"], h1, start=True, stop=True)
                h2 = kp.tile([128, 512], dt.bfloat16, tag="h2", bufs=2,
                             name=f"ch2_{nn}")
                nc.scalar.activation(h2, ps, AF.Relu, bias=C["cm_b2"])
                for jj in range(4):
                    t = 4 * nn + jj
                    ps = psm.tile([128, 128], dt.float32, tag="m",
                                  name=f"cm3_{nn}_{jj}")
                    nc.tensor.matmul(ps, h2[:, 128 * jj:128 * (jj + 1)],
                                     C["cm_w3t"], start=True, stop=True)
                    act = nc.scalar.activation(
                        cpre_full[:, 128 * t:128 * (t + 1)], ps, AF.Copy,
                        scale=C["colsc"][:, t:t + 1])
                    if batches > 0:
                        act.wait_op(loc2_sem, 112 * batches, "sem-ge")
                if nn % 2 == 1:
                    # fat sends after chunks 1 and 3 (halves of slot 0)
                    half = nn // 2
                    ag_send(cpre_full, NCL, 1024 * half, 1024,
                            ag2_sem, loc2_sem)

            def lstm_c(nn, lcsn):
                sl = slice(512 * nn, 512 * (nn + 1))
                gts = []
                for g in range(4):
                    gs = slice(128 * g, 128 * (g + 1))
                    ps = psg.tile([128, 512], dt.float32, tag=f"g{g % 2}",
                                  name=f"cg_{nn}_{g}")
                    nc.tensor.matmul(ps, C["cu_wt"][:, gs], lcsn,
                                     start=True, stop=False,
                                     skip_group_check=True)
                    nc.tensor.matmul(ps, C["cu_ut"][:, gs], Ch[:, sl],
                                     start=False, stop=True,
                                     skip_group_check=True)
                    gt = kp.tile([128, 512], dt.bfloat16, tag=f"gate{g}",
                                 bufs=1, name=f"cgt_{nn}_{g}")
                    nc.scalar.activation(gt, ps,
                                         AF.Tanh if g == 2 else AF.Sigmoid,
                                         bias=C["cu_b"][:, g:g + 1])
                    gts.append(gt)
                t1 = kp.tile([128, 512], dt.float32, tag="t1", bufs=1,
                             name=f"ct1_{nn}")
                t2 = kp.tile([128, 512], dt.float32, tag="t2", bufs=1,
                             name=f"ct2_{nn}")
                nc.vector.tensor_tensor(out=t1, in0=gts[1], in1=Cc[:, sl],
                                        op=ALU.mult)
                nc.vector.tensor_tensor(out=t2, in0=gts[0], in1=gts[2],
                                        op=ALU.mult)
                nc.vector.tensor_tensor(out=Cc[:, sl], in0=t1, in1=t2,
                                        op=ALU.add)
                t3 = kp.tile([128, 512], dt.float32, tag="t1", bufs=1,
                             name=f"ct3_{nn}")
                nc.scalar.activation(t3, Cc[:, sl], AF.Tanh)
                nc.vector.tensor_tensor(out=Ch[:, sl], in0=gts[3], in1=t3,
                                        op=ALU.mult)

            def lstm_l(nn, clsn, Lh_src, Lh_dst):
                sl = slice(512 * nn, 512 * (nn + 1))
                flip = slice(512 * (1 - nn), 512 * (2 - nn))
                gts = []
                for g in range(4):
                    gs = slice(128 * g, 128 * (g + 1))
                    ps = psg.tile([128, 512], dt.float32, tag=f"g{g % 2}",
                                  name=f"lg_{nn}_{g}")
                    nc.tensor.matmul(ps, C["lu_wcl"][:, gs], clsn,
                                     start=True, stop=False,
                                     skip_group_check=True)
                    nc.tensor.matmul(ps, C["lu_wfl"][:, gs], Lh_src[:, flip],
                                     start=False, stop=False,
                                     skip_group_check=True)
                    nc.tensor.matmul(ps, C["lu_ut"][:, gs], Lh_src[:, sl],
                                     start=False, stop=True,
                                     skip_group_check=True)
                    gt = kp.tile([128, 512], dt.bfloat16, tag=f"gate{g}",
                                 bufs=1, name=f"lgt_{nn}_{g}")
                    nc.scalar.activation(gt, ps,
                                         AF.Tanh if g == 2 else AF.Sigmoid,
                                         bias=C["lu_b"][:, g:g + 1])
                    gts.append(gt)
                t1 = kp.tile([128, 512], dt.float32, tag="t1", bufs=1,
                             name=f"lt1_{nn}")
                t2 = kp.tile([128, 512], dt.float32, tag="t2", bufs=1,
                             name=f"lt2_{nn}")
                nc.vector.tensor_tensor(out=t1, in0=gts[1], in1=Lc[:, sl],
                                        op=ALU.mult)
                nc.vector.tensor_tensor(out=t2, in0=gts[0], in1=gts[2],
                                        op=ALU.mult)
                nc.vector.tensor_tensor(out=Lc[:, sl], in0=t1, in1=t2,
                                        op=ALU.add)
                t3 = kp.tile([128, 512], dt.float32, tag="t1", bufs=1,
                             name=f"lt3_{nn}")
                nc.scalar.activation(t3, Lc[:, sl], AF.Tanh)
                nc.vector.tensor_tensor(out=Lh_dst[:, sl], in0=gts[3], in1=t3,
                                        op=ALU.mult)

            # ---- prologue: L message of round 0 ----
            for m in range(2):
                l_msg_chunk(Lh_pp[0], m, 0)

            for r in range(rounds):
                Lh = Lh_pp[r % 2]
                Lh_new = Lh_pp[(r + 1) % 2]

                # stream DMAs for this round (issue order = consume order)
                stb = {}
                for d in range(PIN, NCORES):
                    for h in range(2):
                        t = st.tile([128, 8192], dt.float8e4, tag="st",
                                    name=f"b1s{d}h{h}_r{r}")
                        nc.sync.dma_start(out=t, in_=b1.ap()[d, h])
                        stb[("b1", d, h)] = t
                for d in range(NCORES):
                    for h in range(2):
                        t = st.tile([128, 8192], dt.float8e4, tag="st",
                                    name=f"b2s{d}h{h}_r{r}")
                        nc.sync.dma_start(out=t, in_=b2.ap()[d, h])
                        stb[("b2", d, h)] = t

                # ===== dir-1: LC = B^T @ lpre (slot-major, local first) ====
                ps1 = [psd.tile([128, 512], dt.float32, tag=f"d{nn}",
                                name=f"ps1_{r}_{nn}") for nn in range(4)]
                for d in range(NCORES):
                    for nn in range(4):
                        for j in range(4):
                            lhsT = lpre_full[:, NL * d + 256 * j:
                                             NL * d + 256 * (j + 1)] \
                                .rearrange("p (e f) -> p e f", e=2)
                            if d < PIN:
                                rhs = b1pin[d][:, 4096 * nn + 1024 * j:
                                               4096 * nn + 1024 * (j + 1)]
                            else:
                                rhs = stb[("b1", d, nn // 2)][
                                    :, 4096 * (nn % 2) + 1024 * j:
                                    4096 * (nn % 2) + 1024 * (j + 1)]
                            rhs = rhs.rearrange("p (e c) -> p e c", e=2)
                            mm = nc.tensor.matmul(
                                ps1[nn], lhsT, rhs,
                                start=(d == 0 and j == 0),
                                stop=(d == NCORES - 1 and j == 3),
                                perf_mode=mybir.MatmulPerfMode.DoubleRow,
                                skip_group_check=True)
                            if d == 1:
                                # attach twice: move_matmul_waits keeps one
                                # on the MM and moves the rest to the LDW,
                                # which reads the remote lpre as weights
                                for _ in range(3):
                                    mm.wait_op(ag1_sem, 14 * (r + 1),
                                               "sem-ge", check=False)

                # ===== C side =====
                for nn in range(4):
                    sl = slice(512 * nn, 512 * (nn + 1))
                    tmp = kp.tile([128, 512], dt.bfloat16, tag="tmp", bufs=2,
                                  name=f"ctmp_{r}_{nn}")
                    nc.vector.tensor_tensor(out=tmp, in0=ps1[nn],
                                            in1=C["colb"][:, sl], op=ALU.mult)
                    lcsn = kp.tile([128, 512], dt.bfloat16, tag="lcs", bufs=2,
                                   name=f"lcs_{r}_{nn}")
                    nc.vector.tensor_tensor(out=lcsn, in0=tmp,
                                            in1=C["g1t"][:, sl], op=ALU.add)
                    lstm_c(nn, lcsn)
                    c_msg_chunk(nn, 2 * r)

                # ===== dir-2: CL = B @ cpre (slot-major, local first) =====
                ps2 = [psd.tile([128, 512], dt.float32, tag=f"d{nn}",
                                name=f"ps2_{r}_{nn}") for nn in range(2)]
                for d in range(NCORES):
                    for h in range(2):
                        buf = stb[("b2", d, h)]
                        for j4 in range(4):
                            j = 4 * h + j4
                            lhsT = cpre_full[:, NCL * d + 256 * j:
                                             NCL * d + 256 * (j + 1)] \
                                .rearrange("p (e f) -> p e f", e=2)
                            for nn in range(2):
                                rhs = buf[:, 2048 * j4 + 1024 * nn:
                                          2048 * j4 + 1024 * (nn + 1)] \
                                    .rearrange("p (e c) -> p e c", e=2)
                                mm = nc.tensor.matmul(
                                    ps2[nn], lhsT, rhs,
                                    start=(d == 0 and j == 0),
                                    stop=(d == NCORES - 1 and j == 7),
                                    perf_mode=mybir.MatmulPerfMode.DoubleRow,
                                    skip_group_check=True)
                                if d == 1:
                                    for _ in range(3):
                                        mm.wait_op(ag2_sem, 28 * (r + 1),
                                                   "sem-ge", check=False)

                # ===== L side =====
                for nn in range(2):
                    sl = slice(512 * nn, 512 * (nn + 1))
                    tmp = kp.tile([128, 512], dt.bfloat16, tag="tmp", bufs=2,
                                  name=f"ltmp_{r}_{nn}")
                    nc.vector.tensor_tensor(out=tmp, in0=ps2[nn],
                                            in1=C["rowb"][:, sl], op=ALU.mult)
                    clsn = kp.tile([128, 512], dt.bfloat16, tag="lcs", bufs=2,
                                   name=f"cls_{r}_{nn}")
                    nc.vector.tensor_tensor(out=clsn, in0=tmp,
                                            in1=C["g2t"][:, sl], op=ALU.add)
                    lstm_l(nn, clsn, Lh, Lh_new)
                    if r < rounds - 1:
                        l_msg_chunk(Lh_new, nn, r + 1)

            # ===== vote MLP (bias of last layer added host-side) =====
            Lh_fin = Lh_pp[rounds % 2]
            vote_sb = wp.tile([1, NL], dt.float32, name="vote_sb")
            for nn in range(2):
                sl = slice(512 * nn, 512 * (nn + 1))
                ps = psm.tile([128, 512], dt.float32, tag="m", name=f"v1_{nn}")
                nc.tensor.matmul(ps, C["lv_w1t"], Lh_fin[:, sl],
                                 start=True, stop=True)
                h1 = kp.tile([128, 512], dt.bfloat16, tag="h1", bufs=2,
                             name=f"vh1_{nn}")
                nc.scalar.activation(h1, ps, AF.Relu, bias=C["lv_b1"])
                ps = psm.tile([128, 512], dt.float32, tag="m", name=f"v2_{nn}")
                nc.tensor.matmul(ps, C["lv_w2t"], h1, start=True, stop=True)
                h2 = kp.tile([128, 512], dt.bfloat16, tag="h2", bufs=2,
                             name=f"vh2_{nn}")
                nc.scalar.activation(h2, ps, AF.Relu, bias=C["lv_b2"])
                ps = psm.tile([1, 512], dt.float32, tag="m", name=f"v3_{nn}")
                nc.tensor.matmul(ps, C["lv_w3t"], h2, start=True, stop=True)
                nc.scalar.activation(vote_sb[0:1, sl], ps, AF.Copy)
            nc.scalar.dma_start(out=vote_out.ap(), in_=vote_sb)

    nc.compile()
    return nc


# ---------------------------------------------------------------------------
# host-side input preparation
# ---------------------------------------------------------------------------

def prep_inputs(inputs):
    g = {k: np.asarray(v) for k, v in inputs.items()}
    lit_idx = g["lit_idx"].astype(np.int64)
    clause_idx = g["clause_idx"].astype(np.int64)

    B = np.zeros((NL_TOT, NCL_TOT), np.bool_)
    B[lit_idx, clause_idx] = True
    degc = B.sum(0).astype(np.float32)
    degl = B.sum(1).astype(np.float32)
    col = (np.float32(1.0) / (np.sqrt(degc) + np.float32(1e-6))).astype(np.float32)
    row = (np.float32(1.0) / (np.sqrt(degl) + np.float32(1e-6))).astype(np.float32)
    col = np.where(degc > 0, col, np.float32(0)).astype(np.float32)
    row = np.where(degl > 0, row, np.float32(0)).astype(np.float32)

    lit_order = np.concatenate(
        [np.concatenate([np.arange(512 * k, 512 * (k + 1)),
                         NV + np.arange(512 * k, 512 * (k + 1))])
         for k in range(NCORES)])
    Bu = B.astype(np.uint8) * FP8_ONE
    Bp = Bu[lit_order]                      # [8192, 16384] permuted rows
    row_p = row[lit_order]

    Bf32 = B.astype(np.float32)
    s_c = row @ Bf32                        # [NCL_TOT]
    scol_old = (col * s_c).astype(np.float32)
    s_l = (Bf32 @ col)[lit_order]
    srow_old = (row_p * s_l).astype(np.float32)

    def b(x):
        return np.ascontiguousarray(np.asarray(x, np.float32)).astype(bf16)

    common = {
        "lm_w1t": b(g["lm_w1"].T), "lm_w2t": b(g["lm_w2"].T),
        "lm_w3t": b(g["lm_w3"].T),
        "cm_w1t": b(g["cm_w1"].T), "cm_w2t": b(g["cm_w2"].T),
        "cm_w3t": b(g["cm_w3"].T),
        "lv_w1t": b(g["lv_w1"].T), "lv_w2t": b(g["lv_w2"].T),
        "lv_w3t": b(g["lv_w3"].T),
        "lm_b1": np.asarray(g["lm_b1"], np.float32).reshape(128, 1),
        "lm_b2": np.asarray(g["lm_b2"], np.float32).reshape(128, 1),
        "cm_b1": np.asarray(g["cm_b1"], np.float32).reshape(128, 1),
        "cm_b2": np.asarray(g["cm_b2"], np.float32).reshape(128, 1),
        "lv_b1": np.asarray(g["lv_b1"], np.float32).reshape(128, 1),
        "lv_b2": np.asarray(g["lv_b2"], np.float32).reshape(128, 1),
        "cu_wt": b(g["cu_wih"].T), "cu_ut": b(g["cu_whh"].T),
        "lu_wcl": b(g["lu_wih"][:, :D].T), "lu_wfl": b(g["lu_wih"][:, D:].T),
        "lu_ut": b(g["lu_whh"].T),
        "cu_b": np.asarray(g["cu_bih"] + g["cu_bhh"],
                           np.float32).reshape(4, 128).T.copy(),
        "lu_b": np.asarray(g["lu_bih"] + g["lu_bhh"],
                           np.float32).reshape(4, 128).T.copy(),
        "lh0": np.ascontiguousarray(np.broadcast_to(
            np.asarray(g["L_init_w"][:, 0] + g["L_init_b"],
                       np.float32)[:, None], (128, NL))).astype(bf16),
        "ch0": np.ascontiguousarray(np.broadcast_to(
            np.asarray(g["C_init_w"][:, 0] + g["C_init_b"],
                       np.float32)[:, None], (128, NCL))).astype(bf16),
    }
    lm_b3 = np.asarray(g["lm_b3"], np.float32)
    cm_b3 = np.asarray(g["cm_b3"], np.float32)

    in_maps = []
    for r in range(NCORES):
        lsl = slice(NL * r, NL * (r + 1))
        csl = slice(NCL * r, NCL * (r + 1))
        xor_idx = [r ^ d for d in range(NCORES)]

        # b1: [d, h, p, nn2*4096 + j*1024 + e*512 + c]
        #  = Bp[1024*(r^d) + 128*(2j+e) + p, 2048*r + 512*(2h+nn2) + c]
        X = Bp[:, csl]
        Xs = X.reshape(8, 1024, 2048)[xor_idx]
        b1r = np.ascontiguousarray(
            Xs.reshape(8, 4, 2, 128, 2, 2, 512)
            .transpose(0, 4, 3, 5, 1, 2, 6)).reshape(8, 2, 128, 8192).view(f8)

        # b2: [d, h, p, j4*2048 + nn*1024 + e*512 + l]
        #  = Bp[1024*r + 512*nn + l, 2048*(r^d) + 128*(2*(4h+j4)+e) + p]
        Y = Bp[lsl, :].T
        Ys = Y.reshape(8, 2048, 1024)[xor_idx]
        b2r = np.ascontiguousarray(
            Ys.reshape(8, 2, 4, 2, 128, 2, 512)
            .transpose(0, 1, 4, 2, 5, 3, 6)).reshape(8, 2, 128, 8192).view(f8)

        m = dict(common)
        m.update({
            "b1": b1r,
            "b2": b2r,
            "colb": np.ascontiguousarray(np.broadcast_to(
                col[csl][None, :] / GAIN, (128, NCL))).astype(bf16),
            "rowb": np.ascontiguousarray(np.broadcast_to(
                row_p[lsl][None, :] / GAIN, (128, NL))).astype(bf16),
            "g1t": np.ascontiguousarray(
                np.outer(lm_b3, scol_old[csl])).astype(bf16),
            "g2t": np.ascontiguousarray(
                np.outer(cm_b3, srow_old[lsl])).astype(bf16),
            "rowsc": np.ascontiguousarray(
                GAIN * row_p[lsl].reshape(8, 128).T).astype(np.float32),
            "colsc": np.ascontiguousarray(
                GAIN * col[csl].reshape(16, 128).T).astype(np.float32),
        })
        in_maps.append(m)
    return in_maps


def selfcheck_layouts(in_maps, lit_idx, clause_idx):
    """Random probes: device-layout b1/b2 entries vs the raw B matrix."""
    B = np.zeros((NL_TOT, NCL_TOT), np.uint8)
    B[np.asarray(lit_idx), np.asarray(clause_idx)] = FP8_ONE
    lit_order = np.concatenate(
        [np.concatenate([np.arange(512 * k, 512 * (k + 1)),
                         NV + np.arange(512 * k, 512 * (k + 1))])
         for k in range(NCORES)])
    Bp = B[lit_order]
    rng = np.random.default_rng(1)
    for r in (0, 3, 6):
        b1r = in_maps[r]["b1"].view(np.uint8)
        b2r = in_maps[r]["b2"].view(np.uint8)
        for _ in range(60):
            d, h, p = rng.integers(8), rng.integers(2), rng.integers(128)
            nn2, j, e, c = (rng.integers(2), rng.integers(4),
                            rng.integers(2), rng.integers(512))
            off = nn2 * 4096 + j * 1024 + e * 512 + c
            want = Bp[1024 * (r ^ d) + 128 * (2 * j + e) + p,
                      2048 * r + 512 * (2 * h + nn2) + c]
            assert b1r[d, h, p, off] == want, ("b1", r, d, h, p, nn2, j, e, c)
        for _ in range(60):
            d, h, p = rng.integers(8), rng.integers(2), rng.integers(128)
            j4, nn, e, l = (rng.integers(4), rng.integers(2),
                            rng.integers(2), rng.integers(512))
            off = j4 * 2048 + nn * 1024 + e * 512 + l
            want = Bp[1024 * r + 512 * nn + l,
                      2048 * (r ^ d) + 128 * (2 * (4 * h + j4) + e) + p]
            assert b2r[d, h, p, off] == want, ("b2", r, d, h, p, j4, nn, e, l)


_PROGRAM_CACHE = {}


def _get_program(rounds):
    if rounds not in _PROGRAM_CACHE:
        _PROGRAM_CACHE[rounds] = build_program(rounds)
    return _PROGRAM_CACHE[rounds]


def run_device(inputs, trace=False, rounds=None, **kw):
    if rounds is None:
        rounds = int(inputs.get("n_rounds", 16))
    in_maps = prep_inputs(inputs)
    nc = _get_program(rounds)
    res = bass_utils.run_bass_kernel_spmd(
        nc, in_maps, core_ids=list(range(NCORES)), trace=trace, **kw)
    return res


def assemble_votes(res_results, lv_b3):
    votes = np.stack([np.asarray(res_results[k]["vote"]).reshape(NL)
                      for k in range(NCORES)])   # [8, 1024]
    vote = votes + np.float32(lv_b3)
    pos = vote[:, :512].reshape(NV)              # var v -> core v//512
    neg = vote[:, 512:].reshape(NV)
    vj = np.stack([pos, neg], axis=1)            # [4096, 2]
    return vj.reshape(32, -1).mean(axis=1).astype(np.float32)


def kernel(**inputs) -> np.ndarray:
    res = run_device(inputs)
    return assemble_votes(res.results,
                          np.asarray(inputs["lv_b3"]).reshape(-1)[0])
